# revision 5
# baseline (speedup 1.0000x reference)
"""Trainium2 Bass kernel for nn_BasisAffinityGAT (B=8, N=512, D=R=128, K=8).

Math (matches reference.py):
    fused = concat(desc, nve) @ W_fuse + b_fuse                 [B,N,D]
    q = fused @ W_q[k];  kk = fused @ W_k[k]                    per basis
    e_q[b,k,n] = lrelu(q).a_q[k];  e_k likewise
    logits = e_q[:,:,:,None] + e_k[:,:,None,:], symmetrized
    alpha  = softmax(logits, -1); ema update; bias_log = log(clip(ema'))

Exact algebra used:
  * sym-logits[i,j] = 0.5*(s_i + s_j) with s = e_q + e_k, so the row
    softmax collapses: alpha[b,k,i,j] = softmax_j(0.5*s[b,k,:])[j],
    independent of i.
  * lrelu(x) = 0.6*x + 0.4*|x| (slope 0.2), so
    0.5*s[b,k,n] = fused[b,n,:] @ wlin[:,k]
                   + 0.2*(a_q[k] . |q_T|) + 0.2*(a_k[k] . |k_T|)
    with wlin[:,k] = 0.3*(W_q[k] @ a_q[k] + W_k[k] @ a_k[k]) host-folded.
  * bias_log content is batch-independent ([K,N,N] broadcast over B).

Sharding (8 cores, SPMD, zero cross-core communication): core m owns
basis k=m for ALL batches.

v2 performance structure (the kernel is output-write-bound: 16 MiB of
DRAM writes per core vs ~2.8 MB of reads):
  * inputs are cast to bf16 on host (fuse matmul runs bf16; everything
    downstream stays fp32r as in v1) halving the read traffic.
  * per-batch compute is decoupled from the write stream: deep bufs on
    the xb / rep tiles let all 8 batches' compute run ahead while the
    alpha DMAs drain as a backlog on the SP HWDGE ring.
  * bias writes go on the ACT HWDGE ring so they interleave with the
    remaining alpha backlog instead of serializing after it.
  * input reads go on the gpsimd SWDGE ring (xb2..7, ema) so they never
    head-of-line-block the write rings; the launch-critical loads (xb0,
    xb1, weights) ride the two HWDGE rings before any writes queue.
  * pbar (batch-mean of p) is accumulated on a [1,N] psum slice and
    broadcast to 128 partitions once at the end, instead of a [128,N]
    DVE accumulation per batch.
"""

import sys

import numpy as np

if "/opt/trn_rl_repo" not in sys.path:
    sys.path.insert(0, "/opt/trn_rl_repo")

from contextlib import ExitStack

import ml_dtypes

import concourse.bass as bass
import concourse.tile as tile
from concourse import bacc, mybir
from concourse.bass_utils import run_bass_kernel_spmd

B, N, D, K = 8, 512, 128, 8
R = D
MOM = 0.99
EPS = 1e-6
N_CORES = 8
F32 = mybir.dt.float32
F32R = mybir.dt.float32r
BF16 = mybir.dt.bfloat16
AF = mybir.ActivationFunctionType
ALU = mybir.AluOpType
PBAR_C = 0.01 / B / MOM  # (1-MOM)/B scaled so Ln(scale=MOM) folds MOM back


def build():
    """Build the SPMD per-core Bass program (identical on all 8 cores)."""
    nc = bacc.Bacc("TRN2", target_bir_lowering=False, debug=False,
                   num_devices=N_CORES)

    # ---- per-core external tensors -------------------------------------
    # xTall[b,h,d,n]: h=0 desc[b].T, h=1 nve[b].T  (same array on all cores)
    xTall = nc.dram_tensor("xTall", [B, 2, D, N], BF16, kind="ExternalInput")
    wfuse = nc.dram_tensor("wfuse", [2, D, D], BF16, kind="ExternalInput")
    bfuse = nc.dram_tensor("bfuse", [D, 1], F32, kind="ExternalInput")
    wq = nc.dram_tensor("wq", [D, R], F32R, kind="ExternalInput")   # W_q[m]
    wk = nc.dram_tensor("wk", [D, R], F32R, kind="ExternalInput")   # W_k[m]
    aq1 = nc.dram_tensor("aq1", [R, 1], F32R, kind="ExternalInput")
    ak1 = nc.dram_tensor("ak1", [R, 1], F32R, kind="ExternalInput")
    wlin1 = nc.dram_tensor("wlin1", [D, 1], F32R, kind="ExternalInput")
    ema = nc.dram_tensor("ema", [N, N], BF16, kind="ExternalInput")  # [m]
    alpha = nc.dram_tensor("alpha", [B, N, N], F32, kind="ExternalOutput")
    biaso = nc.dram_tensor("bias", [B, N, N], F32, kind="ExternalOutput")

    with ExitStack() as ctx:
        tc = ctx.enter_context(tile.TileContext(nc))
        const = ctx.enter_context(tc.tile_pool(name="const", bufs=1))
        work = ctx.enter_context(tc.tile_pool(name="work", bufs=2))
        absp = ctx.enter_context(tc.tile_pool(name="absp", bufs=4))
        psum = ctx.enter_context(tc.tile_pool(name="psum", bufs=1, space="PSUM"))

        wfuse_sb = const.tile([D, 2 * D], BF16)
        bfuse_sb = const.tile([D, 1], F32)
        wq_sb = const.tile([D, R], F32R)
        wk_sb = const.tile([D, R], F32R)
        aq_sb = const.tile([R, 1], F32R)
        ak_sb = const.tile([R, 1], F32R)
        wlin_sb = const.tile([D, 1], F32R)
        ones1_sb = const.tile([1, D], F32)
        onesc_sb = const.tile([1, D], F32R)
        pbar_sb = const.tile([1, N], F32R)
        ema_sb = const.tile([128, 4 * N], F32)

        # launch-critical loads on the two HWDGE rings (SWDGE's first
        # packet is ~5us late); everything else prefetches via gpsimd.
        xb_tiles = [absp.tile([D, 2 * N], BF16, tag="xb", bufs=B,
                              name=f"xb{b}") for b in range(B)]
        nc.scalar.dma_start(
            xb_tiles[0][:].rearrange("d (h n) -> d h n", h=2),
            xTall[0].rearrange("h d n -> d h n"))
        nc.sync.dma_start(wfuse_sb[:].rearrange("d (h c) -> d h c", h=2),
                          wfuse.ap().rearrange("h d c -> d h c"))
        nc.sync.dma_start(bfuse_sb[:], bfuse[:])
        nc.sync.dma_start(wq_sb[:], wq[:])
        nc.sync.dma_start(wk_sb[:], wk[:])
        nc.sync.dma_start(aq_sb[:], aq1[:])
        nc.sync.dma_start(ak_sb[:], ak1[:])
        nc.sync.dma_start(wlin_sb[:], wlin1[:])
        nc.scalar.dma_start(
            xb_tiles[1][:].rearrange("d (h n) -> d h n", h=2),
            xTall[1].rearrange("h d n -> d h n"))
        for b in range(2, B):
            nc.gpsimd.dma_start(
                xb_tiles[b][:].rearrange("d (h n) -> d h n", h=2),
                xTall[b].rearrange("h d n -> d h n"))
        # ema: bf16 in DRAM, cast to f32 during the SWDGE transfer
        nc.gpsimd.dma_start(
            ema_sb[:].rearrange("p (c n) -> p c n", c=4),
            ema.ap().rearrange("(c p) n -> p c n", p=128))
        nc.vector.memset(ones1_sb[:], 1.0)
        nc.vector.tensor_copy(onesc_sb[:], ones1_sb[:])  # f32r copy of ones

        for b in range(B):
            xb = xb_tiles[b]
            psum_f = psum.tile([D, N], F32, tag="mm", bufs=4)
            nc.tensor.matmul(psum_f[:], wfuse_sb[:, 0:D], xb[:, 0:N],
                             start=True, stop=False)
            nc.tensor.matmul(psum_f[:], wfuse_sb[:, D:2 * D],
                             xb[:, N:2 * N], start=False, stop=True)
            fused_sb = absp.tile([D, N], F32R, tag="fused", bufs=3)
            nc.vector.tensor_scalar_add(fused_sb[:], psum_f[:], bfuse_sb[:])
            psum_s = psum.tile([1, N], F32, tag="ps", bufs=2)
            nc.tensor.matmul(psum_s[:], wlin_sb[:], fused_sb[:],
                             start=True, stop=False)
            psum_q = psum.tile([D, N], F32, tag="mm", bufs=4)
            nc.tensor.matmul(psum_q[:], wq_sb[:], fused_sb[:],
                             start=True, stop=True)
            absq = absp.tile([D, N], F32R, tag="abs", bufs=4)
            nc.scalar.activation(absq[:], psum_q[:], AF.Abs)
            nc.tensor.matmul(psum_s[:], aq_sb[:], absq[:],
                             start=False, stop=False)
            psum_k = psum.tile([D, N], F32, tag="mm", bufs=4)
            nc.tensor.matmul(psum_k[:], wk_sb[:], fused_sb[:],
                             start=True, stop=True)
            absk = absp.tile([D, N], F32R, tag="abs", bufs=4)
            nc.scalar.activation(absk[:], psum_k[:], AF.Abs)
            nc.tensor.matmul(psum_s[:], ak_sb[:], absk[:],
                             start=False, stop=True)

            # ---- softmax over free dim (no max-shift: |s| is O(1), exp
            # is safe in fp32 and softmax is shift-invariant) -------------
            expv = work.tile([1, N], F32R, tag="ex", bufs=6)
            sume = work.tile([1, 1], F32, tag="se", bufs=6)
            nc.scalar.activation(expv[:], psum_s[:], AF.Exp,
                                 scale=1.0, accum_out=sume[:])
            rsum = work.tile([1, 1], F32, tag="rs", bufs=6)
            nc.vector.reciprocal(rsum[:], sume[:])

            # pbar += PBAR_C * p  on a single [1,N] lane
            rsum_c = work.tile([1, 1], F32, tag="rc", bufs=6)
            nc.vector.tensor_scalar_mul(rsum_c[:], rsum[:], PBAR_C)
            if b == 0:
                nc.vector.tensor_scalar(pbar_sb[:], expv[:], rsum_c[:], None,
                                        op0=ALU.mult)
            else:
                nc.vector.scalar_tensor_tensor(
                    pbar_sb[:], expv[:], rsum_c[:], pbar_sb[:],
                    op0=ALU.mult, op1=ALU.add)

            # ---- alpha[b, i, :] = p_b for all i ------------------------
            # broadcast via PE: lhsT = rsum replicated (fp32r) so the
            # matmul computes rsum*expv = p on all 128 partitions.
            rsum_rep = work.tile([1, D], F32R, tag="rr", bufs=6)
            nc.vector.tensor_scalar_mul(rsum_rep[:], ones1_sb[:], rsum[:])
            psum_rep = psum.tile([128, N], F32, tag="rep", bufs=2)
            nc.tensor.matmul(psum_rep[:], rsum_rep[:], expv[:],
                             start=True, stop=True)
            rep_t = work.tile([128, N], F32, tag="repsb", bufs=B)
            nc.vector.tensor_copy(rep_t[:], psum_rep[:])
            src = rep_t[:].rearrange(
                "p (o n) -> p o n", o=1).broadcast_to([128, 4, N])
            dst = alpha[b].rearrange("(p i) j -> p i j", p=128)
            nc.sync.dma_start(dst, src)

        # ---- bias_log: broadcast pbar once, then 4 row-chunks ----------
        psum_pb = psum.tile([128, N], F32, tag="rep", bufs=2)
        nc.tensor.matmul(psum_pb[:], onesc_sb[:], pbar_sb[:],
                         start=True, stop=True)
        for c in range(4):
            u = work.tile([128, N], F32, tag="u", bufs=2)
            nc.vector.tensor_add(u[:], ema_sb[:, bass.ts(c, N)], psum_pb[:])
            v = work.tile([128, N], F32, tag="v", bufs=2)
            nc.vector.tensor_scalar_max(v[:], u[:], EPS / MOM)
            bias_t = work.tile([128, N], F32, tag="biassb", bufs=2)
            nc.scalar.activation(bias_t[:], v[:], AF.Ln, scale=MOM)
            src = bias_t[:].rearrange(
                "p (o n) -> p o n", o=1).broadcast_to([128, B, N])
            dst = biaso.ap().rearrange("b (c p) j -> c p b j", c=4)[c]
            nc.scalar.dma_start(dst, src)

    nc.compile()
    return nc


_NC_CACHE = None


def _get_nc():
    global _NC_CACHE
    if _NC_CACHE is None:
        _NC_CACHE = build()
    return _NC_CACHE


def make_in_maps(desc_embeddings, name_value_embeddings, W_fuse, b_fuse,
                 W_q, W_k, a, alpha_ema):
    """Host-side sharding / weight prep -> per-core input dicts."""
    bf16 = ml_dtypes.bfloat16
    desc = np.asarray(desc_embeddings, np.float32)
    nve = np.asarray(name_value_embeddings, np.float32)
    W_fuse = np.asarray(W_fuse, np.float32)
    b_fuse = np.asarray(b_fuse, np.float32)
    W_q = np.asarray(W_q, np.float32)
    W_k = np.asarray(W_k, np.float32)
    a = np.asarray(a, np.float32)
    alpha_ema = np.asarray(alpha_ema, np.float32)

    a_q = a[:, :R, 0]                      # [K,R]
    a_k = a[:, R:, 0]                      # [K,R]
    wlin = 0.3 * (np.einsum("kdr,kr->kd", W_q, a_q)
                  + np.einsum("kdr,kr->kd", W_k, a_k))  # [K,D]

    # xTall[b] = [desc[b].T, nve[b].T] — shared across cores, bf16
    xTall = np.ascontiguousarray(
        np.stack([np.stack([desc[b].T, nve[b].T], axis=0)
                  for b in range(B)], axis=0)).astype(bf16)
    wfuse_stack = np.ascontiguousarray(W_fuse.reshape(2, D, D)).astype(bf16)
    bfuse_col = np.ascontiguousarray(b_fuse.reshape(D, 1))

    shared = dict(xTall=xTall, wfuse=wfuse_stack, bfuse=bfuse_col)
    in_maps = []
    for m in range(N_CORES):
        in_maps.append(dict(
            shared,
            wq=np.ascontiguousarray(W_q[m]),
            wk=np.ascontiguousarray(W_k[m]),
            aq1=np.ascontiguousarray(0.2 * a_q[m].reshape(R, 1)),
            ak1=np.ascontiguousarray(0.2 * a_k[m].reshape(R, 1)),
            wlin1=np.ascontiguousarray(wlin[m].reshape(R, 1)),
            ema=np.ascontiguousarray(alpha_ema[m]).astype(bf16)))
    return in_maps


def gather(results):
    alpha_full = np.stack([r["alpha"] for r in results], axis=1)
    bias_full = np.stack([r["bias"] for r in results], axis=1)
    return bias_full, alpha_full


def kernel(**inputs):
    nc = _get_nc()
    in_maps = make_in_maps(**inputs)
    res = run_bass_kernel_spmd(nc, in_maps, list(range(N_CORES)))
    return gather(res.results)
